# revision 1
# baseline (speedup 1.0000x reference)
"""Single-head attention (B=16, S=1024, D=768) on 8 Trainium2 NeuronCores.

Sharding: data-parallel over batch — each core computes 2 full batches
(QKV projection, S = q@k^T, softmax, P@V, output projection) with all
weights replicated. No collectives.

Layout strategy (all matmul operands float32r — full PE rate at N>=256,
~tf32 accuracy, PE rounds raw fp32 bits internally so no rounding passes):
  - x is host-transposed to xT [d, t] so the d-contraction runs directly.
  - q, k are produced transposed ([d, t]); v token-major ([t, d]).
  - S is computed TRANSPOSED ([j, i] = keys on partitions) so exp(S) lands
    directly in the layout P@V needs — no on-chip transpose of P.
  - softmax denominator via a ones-vector matmul on the PE (column sums),
    reciprocal + gpsimd partition_broadcast, normalization on DVE.
  - scale 1/sqrt(D) is folded into w_q/b_q on the host; biases b_q/b_k are
    per-partition ACT bias; b_v is applied after P@V (rows sum to 1) as a
    per-partition ACT bias; b_out is a rank-1 ones x b_out matmul into PSUM.
"""

import sys

import numpy as np

if "/opt/trn_rl_repo" not in sys.path:
    sys.path.insert(0, "/opt/trn_rl_repo")

import concourse.bass as bass  # noqa: E402
import concourse.mybir as mybir  # noqa: E402
import concourse.tile as tile  # noqa: E402
from concourse import bacc  # noqa: E402
from concourse.bass_interp import get_hw_module  # noqa: E402
from concourse.bass_utils import run_bass_kernel_spmd  # noqa: E402

N_CORES = 8
B, S, D = 16, 1024, 768
BL = B // N_CORES  # batches per core
E3 = 3 * D
KT = D // 128  # 6 contraction tiles
F32 = mybir.dt.float32
F32R = mybir.dt.float32r

_prog = None


def _build():
    nc = bacc.Bacc("TRN2", target_bir_lowering=False, debug=False, num_devices=N_CORES)
    xT_d = nc.dram_tensor("xT", [BL, D, S], F32R, kind="ExternalInput").ap()
    wqkv_d = nc.dram_tensor("wqkvT", [D, E3], F32R, kind="ExternalInput").ap()
    wout_d = nc.dram_tensor("woutT", [D, D], F32R, kind="ExternalInput").ap()
    bqkv_d = nc.dram_tensor("bqkv", [128, 18], F32, kind="ExternalInput").ap()
    bout_d = nc.dram_tensor("bout", [1, D], F32R, kind="ExternalInput").ap()
    onc_d = nc.dram_tensor("ones_col", [128, 1], F32R, kind="ExternalInput").ap()
    onr_d = nc.dram_tensor("ones_row", [1, 128], F32R, kind="ExternalInput").ap()
    y_d = nc.dram_tensor("y", [BL, S, D], F32, kind="ExternalOutput").ap()

    Exp = mybir.ActivationFunctionType.Exp
    Ident = mybir.ActivationFunctionType.Identity
    wqkv_t = wqkv_d.rearrange("(ko p) e -> p ko e", p=128)

    with tile.TileContext(nc) as tc:
        with tc.tile_pool(name="consts", bufs=1) as consts, \
             tc.tile_pool(name="wqk", bufs=3) as wqkp, \
             tc.tile_pool(name="wv", bufs=1) as wvp, \
             tc.tile_pool(name="xT", bufs=1) as xp, \
             tc.tile_pool(name="qk", bufs=1) as qkp, \
             tc.tile_pool(name="v", bufs=1) as vp, \
             tc.tile_pool(name="pt", bufs=1) as ptp, \
             tc.tile_pool(name="ot", bufs=1) as otp, \
             tc.tile_pool(name="y", bufs=2) as yp, \
             tc.tile_pool(name="small", bufs=2) as smallp, \
             tc.tile_pool(name="mm", bufs=4, space="PSUM") as mmp, \
             tc.tile_pool(name="row", bufs=2, space="PSUM") as rowp:

            w_out_sb = consts.tile([128, KT, D], F32R)
            nc.sync.dma_start(w_out_sb[:], wout_d.rearrange("(ko p) e -> p ko e", p=128))
            b_sb = consts.tile([128, 18], F32)
            nc.sync.dma_start(b_sb[:], bqkv_d[:])
            bout_sb = consts.tile([1, D], F32R)
            nc.sync.dma_start(bout_sb[:], bout_d[:])
            onc_sb = consts.tile([128, 1], F32R)
            nc.sync.dma_start(onc_sb[:], onc_d[:])
            onr_sb = consts.tile([1, 128], F32R)
            nc.sync.dma_start(onr_sb[:], onr_d[:])

            for b in range(BL):
                xT = xp.tile([128, KT, S], F32R, tag="xT")
                nc.sync.dma_start(xT[:], xT_d[b].rearrange("(ko p) t -> p ko t", p=128))
                qT = qkp.tile([128, KT, S], F32R, tag="qT")
                kT = qkp.tile([128, KT, S], F32R, tag="kT")
                v_sb = vp.tile([128, 8, D], F32R, tag="v")

                # A-qk: qkvT[e, t] = w_qkv @ x^T for the q/k rows
                for et in range(12):
                    wsl = wqkp.tile([128, KT, 128], F32R, tag="wqk")
                    nc.sync.dma_start(wsl[:], wqkv_t[:, :, 128 * et:128 * (et + 1)])
                    for th in range(2):
                        ps = mmp.tile([128, 512], F32, tag="mm")
                        for kt in range(KT):
                            nc.tensor.matmul(ps[:], wsl[:, kt],
                                             xT[:, kt, 512 * th:512 * (th + 1)],
                                             start=(kt == 0), stop=(kt == KT - 1))
                        dst = qT if et < 6 else kT
                        nc.scalar.activation(dst[:, et % 6, 512 * th:512 * (th + 1)],
                                             ps[:], Ident, bias=b_sb[:, et:et + 1])

                # A-v: v[t, dv] token-major
                for doff, dsz in ((0, 512), (512, 256)):
                    wv = wvp.tile([128, KT, 512], F32R, tag="wv")
                    nc.sync.dma_start(wv[:, :, :dsz],
                                      wqkv_t[:, :, 2 * D + doff:2 * D + doff + dsz])
                    for tt in range(8):
                        ps = mmp.tile([128, 512], F32, tag="mm")
                        for kt in range(KT):
                            nc.tensor.matmul(ps[:, :dsz],
                                             xT[:, kt, 128 * tt:128 * (tt + 1)],
                                             wv[:, kt, :dsz],
                                             start=(kt == 0), stop=(kt == KT - 1))
                        nc.vector.tensor_copy(v_sb[:, tt, doff:doff + dsz], ps[:, :dsz])

                for ih in range(2):
                    # B: S^T[j, i] tiles -> exp -> PT (unnormalized)
                    PT = ptp.tile([128, 8, 512], F32R, tag="PT")
                    for jt in range(8):
                        ps = mmp.tile([128, 512], F32, tag="mm")
                        for dt in range(KT):
                            nc.tensor.matmul(ps[:], kT[:, dt, 128 * jt:128 * (jt + 1)],
                                             qT[:, dt, 512 * ih:512 * (ih + 1)],
                                             start=(dt == 0), stop=(dt == KT - 1))
                        nc.scalar.activation(PT[:, jt], ps[:], Exp)

                    # C: denominator (column sums via ones matmul), normalize PT
                    pr = rowp.tile([1, 512], F32, tag="row")
                    for jt in range(8):
                        nc.tensor.matmul(pr[:], onc_sb[:], PT[:, jt],
                                         start=(jt == 0), stop=(jt == 7))
                    rr = smallp.tile([1, 512], F32, tag="rr")
                    nc.vector.reciprocal(rr[:], pr[:])
                    rb = smallp.tile([128, 512], F32, tag="rb")
                    nc.gpsimd.partition_broadcast(rb[:], rr[:])
                    for jt in range(8):
                        nc.vector.tensor_mul(PT[:, jt], PT[:, jt], rb[:])

                    # D: outT[dv, i] = v^T @ P^T ; bias b_v per partition
                    outT = otp.tile([128, KT, 512], F32R, tag="outT")
                    for dvt in range(KT):
                        ps = mmp.tile([128, 512], F32, tag="mm")
                        for jt in range(8):
                            nc.tensor.matmul(ps[:], v_sb[:, jt, 128 * dvt:128 * (dvt + 1)],
                                             PT[:, jt], start=(jt == 0), stop=(jt == 7))
                        nc.scalar.activation(outT[:, dvt], ps[:], Ident,
                                             bias=b_sb[:, 12 + dvt:13 + dvt])

                    # E: y[t, e] = outT^T @ w_out^T + b_out (rank-1 matmul bias)
                    for tt4 in range(4):
                        tt = 4 * ih + tt4
                        yt = yp.tile([128, D], F32, tag="y")
                        for eoff, esz in ((0, 512), (512, 256)):
                            ps = mmp.tile([128, 512], F32, tag="mm")
                            for dvt in range(KT):
                                nc.tensor.matmul(ps[:, :esz],
                                                 outT[:, dvt, 128 * tt4:128 * (tt4 + 1)],
                                                 w_out_sb[:, dvt, eoff:eoff + esz],
                                                 start=(dvt == 0), stop=False)
                            nc.tensor.matmul(ps[:, :esz], onr_sb[:],
                                             bout_sb[:, eoff:eoff + esz],
                                             start=False, stop=True)
                            nc.vector.tensor_copy(yt[:, eoff:eoff + esz], ps[:, :esz])
                        nc.sync.dma_start(y_d[b, 128 * tt:128 * (tt + 1), :], yt[:])

    nc.compile()
    nc.m = get_hw_module(nc.m)
    return nc


def _prepare_in_maps(x, w_qkv, b_qkv, w_out, b_out):
    x = np.asarray(x, dtype=np.float32)
    w_qkv = np.asarray(w_qkv, dtype=np.float32)
    b_qkv = np.asarray(b_qkv, dtype=np.float32)
    w_out = np.asarray(w_out, dtype=np.float32)
    b_out = np.asarray(b_out, dtype=np.float32)

    s = D ** -0.5
    wq = np.ascontiguousarray(w_qkv.T)  # [D, 3D]
    wq[:, :D] *= s
    bq = b_qkv.copy()
    bq[:D] *= s
    b_arr = np.ascontiguousarray(bq.reshape(18, 128).T)  # [128, 18]
    woutT = np.ascontiguousarray(w_out.T)  # [D, D]
    bout_arr = np.ascontiguousarray(b_out[None, :])
    ones_col = np.ones((128, 1), np.float32)
    ones_row = np.ones((1, 128), np.float32)

    in_maps = []
    for c in range(N_CORES):
        xl = x[BL * c:BL * (c + 1)]
        xT = np.ascontiguousarray(xl.transpose(0, 2, 1))  # [BL, D, S]
        in_maps.append({
            "xT": xT, "wqkvT": wq, "woutT": woutT, "bqkv": b_arr,
            "bout": bout_arr, "ones_col": ones_col, "ones_row": ones_row,
        })
    return in_maps


def _get_prog():
    global _prog
    if _prog is None:
        _prog = _build()
    return _prog


def _run(in_maps, **kwargs):
    res = run_bass_kernel_spmd(_get_prog(), in_maps, list(range(N_CORES)), **kwargs)
    return res


def kernel(x, w_qkv, b_qkv, w_out, b_out):
    in_maps = _prepare_in_maps(x, w_qkv, b_qkv, w_out, b_out)
    res = _run(in_maps)
    y = np.concatenate([res.results[c]["y"] for c in range(N_CORES)], axis=0)
    return y.astype(np.float32)


# revision 2
# speedup vs baseline: 1.1550x; 1.1550x over previous
"""Single-head attention (B=16, S=1024, D=768) on 8 Trainium2 NeuronCores.

Sharding: data-parallel over batch — each core computes 2 full batches
(QKV projection, S = q@k^T, softmax, P@V, output projection) with all
weights replicated. No collectives.

Layout strategy (all matmul operands float32r — full PE rate at N>=256,
~tf32 accuracy, PE rounds raw fp32 bits internally so no rounding passes):
  - x is host-transposed to xT [d, t] so the d-contraction runs directly.
  - q, k are produced transposed ([d, t]); v token-major ([t, d]).
  - S is computed TRANSPOSED ([j, i] = keys on partitions) so exp(S) lands
    directly in the layout P@V needs — no on-chip transpose of P.
  - softmax denominator via a ones-vector matmul on the PE (column sums);
    the normalization is applied to the P@V output (outT), keeping the
    reciprocal/broadcast chain off the PE critical path.
  - scale 1/sqrt(D) is folded into w_q/b_q on the host; biases b_q/b_k are
    per-partition ACT bias during the PSUM->SBUF copy; b_v is folded into
    b_out on the host (b_out_eff = b_out + w_out @ b_v); b_out_eff enters
    via a rank-1 ones x b_out matmul accumulated into the PSUM.
"""

import sys

import numpy as np

if "/opt/trn_rl_repo" not in sys.path:
    sys.path.insert(0, "/opt/trn_rl_repo")

import concourse.bass as bass  # noqa: E402
import concourse.mybir as mybir  # noqa: E402
import concourse.tile as tile  # noqa: E402
from concourse import bacc  # noqa: E402
from concourse.bass_interp import get_hw_module  # noqa: E402
from concourse.bass_utils import run_bass_kernel_spmd  # noqa: E402

N_CORES = 8
B, S, D = 16, 1024, 768
BL = B // N_CORES  # batches per core
E3 = 3 * D
KT = D // 128  # 6 contraction tiles
F32 = mybir.dt.float32
F32R = mybir.dt.float32r

_prog = None


def _build():
    nc = bacc.Bacc("TRN2", target_bir_lowering=False, debug=False, num_devices=N_CORES)
    xT_d = nc.dram_tensor("xT", [BL, D, S], F32R, kind="ExternalInput").ap()
    wqkv_d = nc.dram_tensor("wqkvT", [D, E3], F32R, kind="ExternalInput").ap()
    wout_d = nc.dram_tensor("woutT", [D, D], F32R, kind="ExternalInput").ap()
    bqkv_d = nc.dram_tensor("bqkv", [128, 12], F32, kind="ExternalInput").ap()
    bout_d = nc.dram_tensor("bout", [1, D], F32R, kind="ExternalInput").ap()
    onc_d = nc.dram_tensor("ones_col", [128, 1], F32R, kind="ExternalInput").ap()
    onr_d = nc.dram_tensor("ones_row", [1, 128], F32R, kind="ExternalInput").ap()
    y_d = nc.dram_tensor("y", [BL, S, D], F32, kind="ExternalOutput").ap()

    Exp = mybir.ActivationFunctionType.Exp
    Ident = mybir.ActivationFunctionType.Identity
    Mult = mybir.AluOpType.mult
    wqkv_t = wqkv_d.rearrange("(ko p) e -> p ko e", p=128)
    xT_t = [xT_d[b].rearrange("(ko p) t -> p ko t", p=128) for b in range(BL)]

    with tile.TileContext(nc) as tc:
        with tc.tile_pool(name="consts", bufs=1) as consts, \
             tc.tile_pool(name="wqk", bufs=2) as wqkp, \
             tc.tile_pool(name="wv", bufs=1) as wvp, \
             tc.tile_pool(name="xT", bufs=3) as xp, \
             tc.tile_pool(name="qk", bufs=1) as qkp, \
             tc.tile_pool(name="v", bufs=1) as vp, \
             tc.tile_pool(name="pt", bufs=2) as ptp, \
             tc.tile_pool(name="ot", bufs=1) as otp, \
             tc.tile_pool(name="y", bufs=2) as yp, \
             tc.tile_pool(name="small", bufs=2) as smallp, \
             tc.tile_pool(name="mm", bufs=6, space="PSUM") as mmp, \
             tc.tile_pool(name="row", bufs=2, space="PSUM") as rowp:

            b_sb = consts.tile([128, 12], F32)
            nc.sync.dma_start(b_sb[:], bqkv_d[:])
            bout_sb = consts.tile([1, D], F32R)
            nc.sync.dma_start(bout_sb[:], bout_d[:])
            onc_sb = consts.tile([128, 1], F32R)
            nc.sync.dma_start(onc_sb[:], onc_d[:])
            onr_sb = consts.tile([1, 128], F32R)
            nc.sync.dma_start(onr_sb[:], onr_d[:])
            w_out_sb = consts.tile([128, KT, D], F32R)
            nc.sync.dma_start(w_out_sb[:], wout_d.rearrange("(ko p) e -> p ko e", p=128))

            for b in range(BL):
                # x halves (t in [0,512) and [512,1024)), prefetchable
                xh = []
                for h in range(2):
                    t = xp.tile([128, KT, 512], F32R, tag="xT")
                    nc.sync.dma_start(t[:], xT_t[b][:, :, 512 * h:512 * (h + 1)])
                    xh.append(t)
                qT = qkp.tile([128, KT, S], F32R, tag="qT")
                kT = qkp.tile([128, KT, S], F32R, tag="kT")
                v_sb = vp.tile([128, 8, D], F32R, tag="v")

                # A-qk: qkvT[e, t] = w_qkv @ x^T for the q/k rows
                for et in range(12):
                    wsl = wqkp.tile([128, KT, 128], F32R, tag="wqk")
                    nc.sync.dma_start(wsl[:], wqkv_t[:, :, 128 * et:128 * (et + 1)])
                    for th in range(2):
                        ps = mmp.tile([128, 512], F32, tag="mm")
                        for kt in range(KT):
                            nc.tensor.matmul(ps[:], wsl[:, kt], xh[th][:, kt],
                                             start=(kt == 0), stop=(kt == KT - 1))
                        dst = qT if et < 6 else kT
                        nc.scalar.activation(dst[:, et % 6, 512 * th:512 * (th + 1)],
                                             ps[:], Ident, bias=b_sb[:, et:et + 1])

                # A-v: v[t, dv] token-major
                for doff, dsz in ((0, 512), (512, 256)):
                    wv = wvp.tile([128, KT, 512], F32R, tag="wv")
                    nc.sync.dma_start(wv[:, :, :dsz],
                                      wqkv_t[:, :, 2 * D + doff:2 * D + doff + dsz])
                    for tt in range(8):
                        ps = mmp.tile([128, 512], F32, tag="mm")
                        for kt in range(KT):
                            nc.tensor.matmul(ps[:, :dsz],
                                             xh[tt // 4][:, kt, 128 * (tt % 4):128 * (tt % 4 + 1)],
                                             wv[:, kt, :dsz],
                                             start=(kt == 0), stop=(kt == KT - 1))
                        nc.vector.tensor_copy(v_sb[:, tt, doff:doff + dsz], ps[:, :dsz])

                for ih in range(2):
                    # B: S^T[j, i] tiles -> exp -> PT (unnormalized)
                    PT = ptp.tile([128, 8, 512], F32R, tag="PT")
                    for jt in range(8):
                        ps = mmp.tile([128, 512], F32, tag="mm")
                        for dt in range(KT):
                            nc.tensor.matmul(ps[:], kT[:, dt, 128 * jt:128 * (jt + 1)],
                                             qT[:, dt, 512 * ih:512 * (ih + 1)],
                                             start=(dt == 0), stop=(dt == KT - 1))
                        nc.scalar.activation(PT[:, jt], ps[:], Exp)

                    # C: softmax denominator = column sums via ones matmul
                    pr = rowp.tile([1, 512], F32, tag="row")
                    for jt in range(8):
                        nc.tensor.matmul(pr[:], onc_sb[:], PT[:, jt],
                                         start=(jt == 0), stop=(jt == 7))
                    rr = smallp.tile([1, 512], F32, tag="rr")
                    nc.vector.reciprocal(rr[:], pr[:])
                    rb = smallp.tile([128, 512], F32, tag="rb")
                    nc.gpsimd.partition_broadcast(rb[:], rr[:])

                    # D: outT[dv, i] = (v^T @ P^T) * (1/denom) — normalized on DVE
                    outT = otp.tile([128, KT, 512], F32R, tag="outT")
                    for dvt in range(KT):
                        ps = mmp.tile([128, 512], F32, tag="mm")
                        for jt in range(8):
                            nc.tensor.matmul(ps[:], v_sb[:, jt, 128 * dvt:128 * (dvt + 1)],
                                             PT[:, jt], start=(jt == 0), stop=(jt == 7))
                        nc.vector.tensor_tensor(outT[:, dvt], ps[:], rb[:], Mult)

                    # E: y[t, e] = outT^T @ w_out^T + b_out_eff (rank-1 matmul bias)
                    for tt4 in range(4):
                        tt = 4 * ih + tt4
                        yt = yp.tile([128, D], F32, tag="y")
                        for eoff, esz in ((0, 512), (512, 256)):
                            ps = mmp.tile([128, 512], F32, tag="mm")
                            for dvt in range(KT):
                                nc.tensor.matmul(ps[:, :esz],
                                                 outT[:, dvt, 128 * tt4:128 * (tt4 + 1)],
                                                 w_out_sb[:, dvt, eoff:eoff + esz],
                                                 start=(dvt == 0), stop=False)
                            nc.tensor.matmul(ps[:, :esz], onr_sb[:],
                                             bout_sb[:, eoff:eoff + esz],
                                             start=False, stop=True)
                            nc.scalar.copy(yt[:, eoff:eoff + esz], ps[:, :esz])
                        nc.sync.dma_start(y_d[b, 128 * tt:128 * (tt + 1), :], yt[:])

    nc.compile()
    nc.m = get_hw_module(nc.m)
    return nc


def _prepare_in_maps(x, w_qkv, b_qkv, w_out, b_out):
    x = np.asarray(x, dtype=np.float32)
    w_qkv = np.asarray(w_qkv, dtype=np.float32)
    b_qkv = np.asarray(b_qkv, dtype=np.float32)
    w_out = np.asarray(w_out, dtype=np.float32)
    b_out = np.asarray(b_out, dtype=np.float32)

    s = D ** -0.5
    wq = np.ascontiguousarray(w_qkv.T)  # [D, 3D]
    wq[:, :D] *= s
    bqk = b_qkv[:2 * D].copy()
    bqk[:D] *= s
    b_arr = np.ascontiguousarray(bqk.reshape(12, 128).T)  # [128, 12]
    woutT = np.ascontiguousarray(w_out.T)  # [D, D]
    b_out_eff = b_out + w_out @ b_qkv[2 * D:]
    bout_arr = np.ascontiguousarray(b_out_eff[None, :].astype(np.float32))
    ones_col = np.ones((128, 1), np.float32)
    ones_row = np.ones((1, 128), np.float32)

    in_maps = []
    for c in range(N_CORES):
        xl = x[BL * c:BL * (c + 1)]
        xT = np.ascontiguousarray(xl.transpose(0, 2, 1))  # [BL, D, S]
        in_maps.append({
            "xT": xT, "wqkvT": wq, "woutT": woutT, "bqkv": b_arr,
            "bout": bout_arr, "ones_col": ones_col, "ones_row": ones_row,
        })
    return in_maps


def _get_prog():
    global _prog
    if _prog is None:
        _prog = _build()
    return _prog


def _run(in_maps, **kwargs):
    res = run_bass_kernel_spmd(_get_prog(), in_maps, list(range(N_CORES)), **kwargs)
    return res


def kernel(x, w_qkv, b_qkv, w_out, b_out):
    in_maps = _prepare_in_maps(x, w_qkv, b_qkv, w_out, b_out)
    res = _run(in_maps)
    y = np.concatenate([res.results[c]["y"] for c in range(N_CORES)], axis=0)
    return y.astype(np.float32)


# revision 3
# speedup vs baseline: 1.2830x; 1.1108x over previous
"""Single-head attention (B=16, S=1024, D=768) on 8 Trainium2 NeuronCores.

Sharding: data-parallel over batch — each core computes 2 full batches
(QKV projection, S = q@k^T, softmax, P@V, output projection) with all
weights replicated. No collectives.

Layout strategy (all matmul operands float32r — full PE rate at N>=256,
~tf32 accuracy, PE rounds raw fp32 bits internally so no rounding passes):
  - x is host-transposed to xT [d, t] so the d-contraction runs directly.
  - q, k are produced transposed ([d, t]); v token-major ([t, d]).
  - S is computed TRANSPOSED ([j, i] = keys on partitions) so exp(S) lands
    directly in the layout P@V needs — no on-chip transpose of P.
  - softmax denominator via a ones-vector matmul on the PE (column sums);
    the normalization is applied to the P@V output (outT), keeping the
    reciprocal/broadcast chain off the PE critical path.
  - scale 1/sqrt(D) is folded into w_q/b_q on the host; biases b_q/b_k are
    per-partition ACT bias during the PSUM->SBUF copy; b_v is folded into
    b_out on the host (b_out_eff = b_out + w_out @ b_v); b_out_eff enters
    via a rank-1 ones x b_out matmul accumulated into the PSUM.
"""

import sys

import numpy as np

if "/opt/trn_rl_repo" not in sys.path:
    sys.path.insert(0, "/opt/trn_rl_repo")

import concourse.bass as bass  # noqa: E402
import concourse.mybir as mybir  # noqa: E402
import concourse.tile as tile  # noqa: E402
from concourse import bacc  # noqa: E402
from concourse.bass_interp import get_hw_module  # noqa: E402
from concourse.bass_utils import run_bass_kernel_spmd  # noqa: E402

N_CORES = 8
B, S, D = 16, 1024, 768
BL = B // N_CORES  # batches per core
E3 = 3 * D
KT = D // 128  # 6 contraction tiles
F32 = mybir.dt.float32
F32R = mybir.dt.float32r

_prog = None


def _build():
    nc = bacc.Bacc("TRN2", target_bir_lowering=False, debug=False, num_devices=N_CORES)
    xT_d = nc.dram_tensor("xT", [BL, D, S], F32R, kind="ExternalInput").ap()
    wqkv_d = nc.dram_tensor("wqkvT", [D, E3], F32R, kind="ExternalInput").ap()
    wout_d = nc.dram_tensor("woutT", [D, D], F32R, kind="ExternalInput").ap()
    bqkv_d = nc.dram_tensor("bqkv", [128, 12], F32, kind="ExternalInput").ap()
    bout_d = nc.dram_tensor("bout", [128, D], F32, kind="ExternalInput").ap()
    onc_d = nc.dram_tensor("ones_col", [128, 1], F32R, kind="ExternalInput").ap()
    y_d = nc.dram_tensor("y", [BL, S, D], F32, kind="ExternalOutput").ap()

    Exp = mybir.ActivationFunctionType.Exp
    Ident = mybir.ActivationFunctionType.Identity
    Mult = mybir.AluOpType.mult
    wqkv_t = wqkv_d.rearrange("(ko p) e -> p ko e", p=128)
    xT_t = [xT_d[b].rearrange("(ko p) t -> p ko t", p=128) for b in range(BL)]

    with tile.TileContext(nc) as tc:
        with tc.tile_pool(name="consts", bufs=1) as consts, \
             tc.tile_pool(name="wqk", bufs=2) as wqkp, \
             tc.tile_pool(name="wv", bufs=1) as wvp, \
             tc.tile_pool(name="xT", bufs=3) as xp, \
             tc.tile_pool(name="qk", bufs=1) as qkp, \
             tc.tile_pool(name="v", bufs=1) as vp, \
             tc.tile_pool(name="pt", bufs=2) as ptp, \
             tc.tile_pool(name="ot", bufs=1) as otp, \
             tc.tile_pool(name="y", bufs=2) as yp, \
             tc.tile_pool(name="small", bufs=2) as smallp, \
             tc.tile_pool(name="mm", bufs=6, space="PSUM") as mmp, \
             tc.tile_pool(name="row", bufs=2, space="PSUM") as rowp:

            b_sb = consts.tile([128, 12], F32)
            nc.sync.dma_start(b_sb[:], bqkv_d[:])
            bout_sb = consts.tile([128, D], F32)
            nc.sync.dma_start(bout_sb[:], bout_d[:])
            onc_sb = consts.tile([128, 1], F32R)
            nc.sync.dma_start(onc_sb[:], onc_d[:])
            w_out_sb = consts.tile([128, KT, D], F32R)

            for b in range(BL):
                # x halves (t in [0,512) and [512,1024)), prefetchable
                xh = []
                for h in range(2):
                    t = xp.tile([128, KT, 512], F32R, tag="xT")
                    nc.sync.dma_start(t[:], xT_t[b][:, :, 512 * h:512 * (h + 1)])
                    xh.append(t)
                qT = qkp.tile([128, KT, S], F32R, tag="qT")
                kT = qkp.tile([128, KT, S], F32R, tag="kT")
                v_sb = vp.tile([128, 8, D], F32R, tag="v")

                # A-qk: qkvT[e, t] = w_qkv @ x^T for the q/k rows
                for et in range(12):
                    wsl = wqkp.tile([128, KT, 128], F32R, tag="wqk")
                    nc.sync.dma_start(wsl[:], wqkv_t[:, :, 128 * et:128 * (et + 1)])
                    for th in range(2):
                        ps = mmp.tile([128, 512], F32, tag="mm")
                        for kt in range(KT):
                            nc.tensor.matmul(ps[:], wsl[:, kt], xh[th][:, kt],
                                             start=(kt == 0), stop=(kt == KT - 1))
                        dst = qT if et < 6 else kT
                        nc.scalar.activation(dst[:, et % 6, 512 * th:512 * (th + 1)],
                                             ps[:], Ident, bias=b_sb[:, et:et + 1])

                if b == 0:
                    nc.sync.dma_start(w_out_sb[:],
                                      wout_d.rearrange("(ko p) e -> p ko e", p=128))

                # A-v: v[t, dv] token-major
                for doff, dsz in ((0, 512), (512, 256)):
                    wv = wvp.tile([128, KT, 512], F32R, tag="wv")
                    nc.sync.dma_start(wv[:, :, :dsz],
                                      wqkv_t[:, :, 2 * D + doff:2 * D + doff + dsz])
                    for tt in range(8):
                        ps = mmp.tile([128, 512], F32, tag="mm")
                        for kt in range(KT):
                            nc.tensor.matmul(ps[:, :dsz],
                                             xh[tt // 4][:, kt, 128 * (tt % 4):128 * (tt % 4 + 1)],
                                             wv[:, kt, :dsz],
                                             start=(kt == 0), stop=(kt == KT - 1))
                        nc.vector.tensor_copy(v_sb[:, tt, doff:doff + dsz], ps[:, :dsz])

                for ih in range(2):
                    # B: S^T[j, i] tiles -> exp -> PT (unnormalized)
                    PT = ptp.tile([128, 8, 512], F32R, tag="PT")
                    for jt in range(8):
                        ps = mmp.tile([128, 512], F32, tag="mm")
                        for dt in range(KT):
                            nc.tensor.matmul(ps[:], kT[:, dt, 128 * jt:128 * (jt + 1)],
                                             qT[:, dt, 512 * ih:512 * (ih + 1)],
                                             start=(dt == 0), stop=(dt == KT - 1))
                        nc.scalar.activation(PT[:, jt], ps[:], Exp)

                    # C: softmax denominator = column sums via ones matmul
                    pr = rowp.tile([1, 512], F32, tag="row")
                    for jt in range(8):
                        nc.tensor.matmul(pr[:], onc_sb[:], PT[:, jt],
                                         start=(jt == 0), stop=(jt == 7))
                    rr = smallp.tile([1, 512], F32, tag="rr")
                    nc.vector.reciprocal(rr[:], pr[:])
                    rb = smallp.tile([128, 512], F32, tag="rb")
                    nc.gpsimd.partition_broadcast(rb[:], rr[:])

                    # D: outT[dv, i] = (v^T @ P^T) * (1/denom) — normalized on DVE
                    outT = otp.tile([128, KT, 512], F32R, tag="outT")
                    for dvt in range(KT):
                        ps = mmp.tile([128, 512], F32, tag="mm")
                        for jt in range(8):
                            nc.tensor.matmul(ps[:], v_sb[:, jt, 128 * dvt:128 * (dvt + 1)],
                                             PT[:, jt], start=(jt == 0), stop=(jt == 7))
                        nc.vector.tensor_tensor(outT[:, dvt], ps[:], rb[:], Mult)

                    # E: y[t, e] = outT^T @ w_out^T + b_out_eff (rank-1 matmul bias)
                    for tt4 in range(4):
                        tt = 4 * ih + tt4
                        yt = yp.tile([128, D], F32, tag="y")
                        for eoff, esz in ((0, 512), (512, 256)):
                            ps = mmp.tile([128, 512], F32, tag="mm")
                            for dvt in range(KT):
                                nc.tensor.matmul(ps[:, :esz],
                                                 outT[:, dvt, 128 * tt4:128 * (tt4 + 1)],
                                                 w_out_sb[:, dvt, eoff:eoff + esz],
                                                 start=(dvt == 0), stop=(dvt == KT - 1))
                            nc.vector.tensor_tensor(yt[:, eoff:eoff + esz], ps[:, :esz],
                                                    bout_sb[:, eoff:eoff + esz],
                                                    mybir.AluOpType.add)
                        nc.sync.dma_start(y_d[b, 128 * tt:128 * (tt + 1), :], yt[:])

    nc.compile()
    nc.m = get_hw_module(nc.m)
    return nc


def _prepare_in_maps(x, w_qkv, b_qkv, w_out, b_out):
    x = np.asarray(x, dtype=np.float32)
    w_qkv = np.asarray(w_qkv, dtype=np.float32)
    b_qkv = np.asarray(b_qkv, dtype=np.float32)
    w_out = np.asarray(w_out, dtype=np.float32)
    b_out = np.asarray(b_out, dtype=np.float32)

    s = D ** -0.5
    wq = np.ascontiguousarray(w_qkv.T)  # [D, 3D]
    wq[:, :D] *= s
    bqk = b_qkv[:2 * D].copy()
    bqk[:D] *= s
    b_arr = np.ascontiguousarray(bqk.reshape(12, 128).T)  # [128, 12]
    woutT = np.ascontiguousarray(w_out.T)  # [D, D]
    b_out_eff = (b_out + w_out @ b_qkv[2 * D:]).astype(np.float32)
    bout_arr = np.ascontiguousarray(np.broadcast_to(b_out_eff[None, :], (128, D)))
    ones_col = np.ones((128, 1), np.float32)

    in_maps = []
    for c in range(N_CORES):
        xl = x[BL * c:BL * (c + 1)]
        xT = np.ascontiguousarray(xl.transpose(0, 2, 1))  # [BL, D, S]
        in_maps.append({
            "xT": xT, "wqkvT": wq, "woutT": woutT, "bqkv": b_arr,
            "bout": bout_arr, "ones_col": ones_col,
        })
    return in_maps


def _get_prog():
    global _prog
    if _prog is None:
        _prog = _build()
    return _prog


def _run(in_maps, **kwargs):
    res = run_bass_kernel_spmd(_get_prog(), in_maps, list(range(N_CORES)), **kwargs)
    return res


def kernel(x, w_qkv, b_qkv, w_out, b_out):
    in_maps = _prepare_in_maps(x, w_qkv, b_qkv, w_out, b_out)
    res = _run(in_maps)
    y = np.concatenate([res.results[c]["y"] for c in range(N_CORES)], axis=0)
    return y.astype(np.float32)
